# revision 34
# baseline (speedup 1.0000x reference)
"""Causal multi-head attention kernel for TRN2 (8 NeuronCores, SPMD).

Problem: x[2,2048,1024], per-head W_qkv[16,1024,192], W_out[16,64,1024].
  qkv = einsum('bsd,ndh->bnsh', x, W_qkv); causal softmax attention per head;
  out.reshape(B,-1,S); einsum('bds,nhd->bsd', out, W_out).

The final einsum does not contract d, so it reduces to a per-column scale by
W_sum[d] = sum_{n,h} W_out[n,h,d]; done on the host. The device computes the
attention itself.

Sharding: 2 batches x 16 heads = 32 jobs; core c handles batch c//4 and the
4 heads [4*(c%4), 4*(c%4)+4), as 2 head-pairs packed into 128 partitions.

All matmuls fp16 (fp8 fails the error budget: any eps relative jitter on
softmax weights or values becomes ~eps relative output error). The span is
jointly limited by the PE (~86us/core of fp16 streaming) and ScalarE
(~91us/core of exp), so the design goal is keeping BOTH saturated:

  - Q,K projected in column-halves of 512 (8 accumulating matmuls + one
    fp32->fp16 cast each); V projected directly in [s, h] orientation
    (stationary = xT s-chunk, moving = W_v of all 4 heads) straight into
    the AV stationary layout [Va|1|Vb|1] -- no PE transposes.
  - projection work is chopped into ~1-2us thunks and woven between the
    attention k-steps so the first exp lands ~7us in and ScalarE never
    starves while the PE chews projection work.
  - scores: stationary K^T per head [64,128], two heads row-tiled via
    tile_position (0,0)/(64,0) -> concurrent on the PE.  ONE exp per
    k-tile covers both heads ([128, 1024] tile).  Causal column trimming
    (q0) on scores/exp/AV; diagonal-crossing tiles masked by a 0/1 fp16
    multiply on DVE.
  - AV: stationary [V|1] fp16 [128, 65] per head; the ones column makes
    row 64 of the accumulator the softmax denominator.
  - host-packed partition-major DRAM layouts give the DMA contiguous
    multi-KB per-partition lines (full 358 GB/s), so the first projection
    starts ~3us in instead of ~17.
Host epilogue: normalize by the denominator row, reshape, scale by W_sum.
"""

import numpy as np

import concourse.bass as bass
import concourse.mybir as mybir
from concourse.tile import TileContext
from concourse.bass_utils import run_bass_kernel_spmd

F32 = mybir.dt.float32
MMD = mybir.dt.float16
NPD = np.float16

B, S, D, NH, HD = 2, 2048, 1024, 16, 64  # batch, seq, model, heads, head_dim
NCORES = 8
HPC = 4  # heads per core
NPAIR = 2  # head pairs per core
DT = D // 128  # 8 d-tiles
NKT = S // 128  # 16 k tiles
NC4 = S // 512  # 4 column-halves of 512
SCALE = 1.0 / np.sqrt(HD)


def _split_excess_waits(nc, limit=1):
    """This walrus build rejects >1 sync-wait per instruction; hoist extra
    waits onto preceding same-engine no-ops (identical blocking semantics)."""
    cnt = 0
    for fn in nc.m.functions:
        for blk in fn.blocks:
            out = []
            for inst in blk.instructions:
                si = inst.sync_info
                if si is not None and si.on_wait and len(si.on_wait) > limit:
                    waits = list(si.on_wait)
                    excess, keep = waits[:-limit], waits[-limit:]
                    for i in range(0, len(excess), limit):
                        nop = mybir.InstNoOp(
                            name=f"wsplit_{cnt}", ins=[], outs=[], engine=inst.engine
                        )
                        cnt += 1
                        nop.sync_info = mybir.SyncInfo(
                            on_wait=excess[i : i + limit], on_update=[]
                        )
                        out.append(nop)
                    inst.sync_info = mybir.SyncInfo(
                        on_wait=keep, on_update=list(si.on_update or [])
                    )
                out.append(inst)
            blk.instructions = out
    return cnt


def build_nc():
    nc = bass.Bass()
    # host-packed, partition-major layouts (see _host_inputs)
    xt = nc.declare_dram_parameter("xt", [128, NC4, DT, 512], MMD, isOutput=False)
    # rows: [K0(8d), Q0(8d), K1(8d), Q1(8d)]; cols: head a 0:64, head b 64:128
    wqk = nc.declare_dram_parameter("wqk", [128, 4 * DT, 128], MMD, isOutput=False)
    wv = nc.declare_dram_parameter("wv", [128, DT, HPC * HD], MMD, isOutput=False)
    mask = nc.declare_dram_parameter("mask", [128, 4, 1024], MMD, isOutput=False)
    out = nc.declare_dram_parameter("out", [65, HPC * S], F32, isOutput=True)

    with TileContext(nc) as tc:
        with (
            tc.tile_pool(name="persist", bufs=1) as pp,
            tc.tile_pool(name="psum", bufs=2, space="PSUM") as ps,
            tc.tile_pool(name="work", bufs=2) as pc,
        ):
            # ---- persistent SBUF tensors
            qt2 = [pp.tile([128, S], MMD, tag=f"qt{p}", name=f"qtt{p}") for p in range(NPAIR)]
            kt2 = [pp.tile([128, S], MMD, tag=f"kt{p}", name=f"ktt{p}") for p in range(NPAIR)]
            v2e = [
                pp.tile([128, NKT, 130], MMD, tag=f"v2e{p}", name=f"v2e{p}")
                for p in range(NPAIR)
            ]
            mask_sb = pp.tile([128, 4, 1024], MMD, tag="mask", name="mask_sb")
            xt_sb = pp.tile([128, NC4, DT, 512], MMD, tag="xt", name="xt_sb")
            wqk_sb = pp.tile([128, 4 * DT, 128], MMD, tag="wqk", name="wqk_sb")
            wv_sb = pp.tile([128, DT, HPC * HD], MMD, tag="wv", name="wv_sb")

            # DMA split across the two HWDGE queues (SP + Activation) so the
            # x chunks and weights transfer CONCURRENTLY; small leading
            # transfers so the gating semaphores fire early.  The Scalar
            # queue is free until the first exp (~15us), so issuing x DMAs
            # there costs nothing.
            nc.sync.dma_start(out=wqk_sb[:, 0 : 2 * DT, :], in_=wqk[:, 0 : 2 * DT, :])
            nc.scalar.dma_start(out=xt_sb[:, 0, :, :], in_=xt[:, 0, :, :])
            nc.sync.dma_start(out=mask_sb[:], in_=mask[:])
            nc.sync.dma_start(out=wv_sb[:], in_=wv[:])
            nc.sync.dma_start(out=xt_sb[:, 1, :, :], in_=xt[:, 1, :, :])
            nc.sync.dma_start(out=xt_sb[:, 2, :, :], in_=xt[:, 2, :, :])
            nc.sync.dma_start(out=xt_sb[:, 3, :, :], in_=xt[:, 3, :, :])
            nc.sync.dma_start(
                out=wqk_sb[:, 2 * DT : 4 * DT, :], in_=wqk[:, 2 * DT : 4 * DT, :]
            )
            # warmup: PE p-state ramps with ~3us of continuous execution, so
            # burn dummy matmuls on zeroed tiles while the first DMAs land;
            # enough of them that the real matmuls queue with NO idle gap
            # (a >100ns gap drops the PE back to the 1.2GHz p-state).
            zq = pp.tile([128, 128], MMD, tag="zq", name="zq")
            zx = pp.tile([128, 512], MMD, tag="zx", name="zx")
            nc.vector.memset(zq[:], 0.0)
            nc.vector.memset(zx[:], 0.0)
            for _ in range(14):
                wacc = ps.tile([128, 512], F32, tag="acc", name="wacc")
                nc.tensor.matmul(wacc[:], zq[:], zx[:], start=True, stop=True)
            for p in range(NPAIR):
                nc.vector.memset(v2e[p][:, :, 64], 1.0)
                nc.vector.memset(v2e[p][:, :, 129], 1.0)

            def qk_half(p, t, h4):
                """Project one (pair, K/Q) column-half [h4*512, (h4+1)*512).
                t: 0 = K, 1 = Q (matches wqk row packing)."""
                def thunk():
                    acc = ps.tile([128, 512], F32, tag="acc", name="acc")
                    widx = (p * 2 + t) * DT
                    for d in range(DT):
                        nc.tensor.matmul(
                            acc[:],
                            wqk_sb[:, widx + d, :],
                            xt_sb[:, h4, d, :],
                            start=(d == 0),
                            stop=(d == DT - 1),
                        )
                    dst = kt2[p] if t == 0 else qt2[p]
                    nc.vector.tensor_copy(dst[:, h4 * 512 : (h4 + 1) * 512], acc[:])
                return thunk

            def v_chunk(c):
                """Project V for s-chunk c (128 rows) for all 4 heads,
                directly into the [Va|1|Vb|1] AV stationary layout."""
                def thunk():
                    vacc = ps.tile([128, HPC * HD], F32, tag="acc", name="vacc")
                    c4, cs = c // 4, (c % 4) * 128
                    for d in range(DT):
                        nc.tensor.matmul(
                            vacc[:],
                            xt_sb[:, c4, d, cs : cs + 128],
                            wv_sb[:, d, :],
                            start=(d == 0),
                            stop=(d == DT - 1),
                        )
                    for h in range(HPC):
                        nc.vector.tensor_copy(
                            v2e[h // 2][:, c, (h % 2) * 65 : (h % 2) * 65 + 64],
                            vacc[:, h * HD : (h + 1) * HD],
                        )
                return thunk

            def attention(p, qb, feed, pending=None, last=False):
                """One (pair, q-block) attention; thunks from `feed` are
                emitted between k-steps to keep the PE busy while ScalarE
                drains the exps.  `pending` = (tail, epilogue) of the
                PREVIOUS block: its last two AV matmuls and its stage-out
                are deferred into this block, emitted right after this
                block's first scores+exp, so ScalarE crosses the block
                boundary with only one scores-exec of latency.  Returns
                this block's (tail, epilogue)."""
                nk = 4 * (qb + 1)  # causal: k tiles 0..nk-1
                oa = ps.tile([65, 512], F32, tag="acca", name="oa", bufs=1)
                ob = ps.tile([65, 512], F32, tag="accb", name="ob", bufs=1)
                pt_tiles = [None] * nk
                feed = list(feed)

                def scores(k):
                    q0 = max(0, 128 * (k - 4 * qb))
                    s2 = ps.tile([128, 1024], F32, tag="s2", name="s2")
                    qsl = slice(qb * 512 + q0, (qb + 1) * 512)
                    for e in range(2):
                        rows = slice(64 * e, 64 * e + 64)
                        nc.tensor.matmul(
                            s2[:, e * 512 + q0 : (e + 1) * 512],
                            kt2[p][rows, k * 128 : (k + 1) * 128],
                            qt2[p][rows, qsl],
                            start=True,
                            stop=True,
                            tile_position=(64 * e, 0),
                        )
                    pt2 = pc.tile([128, 1024], MMD, tag="pt", name="pt2", bufs=6)
                    nc.scalar.activation(
                        pt2[:, q0:1024],
                        s2[:, q0:1024],
                        mybir.ActivationFunctionType.Exp,
                        scale=float(SCALE),
                    )
                    rel = k - 4 * qb
                    if rel >= 0:  # diagonal-crossing: 0/1 mask
                        nc.vector.tensor_mul(
                            pt2[:, q0:1024],
                            pt2[:, q0:1024],
                            mask_sb[:, rel, q0:1024],
                        )
                    pt_tiles[k] = (pt2, q0)

                def av(k):
                    pt2, q0 = pt_tiles[k]
                    nc.tensor.matmul(
                        oa[:, q0:512],
                        v2e[p][:, k, 0:65],
                        pt2[:, q0:512],
                        start=(k == 0),
                        stop=(k == nk - 1),
                    )
                    nc.tensor.matmul(
                        ob[:, q0:512],
                        v2e[p][:, k, 65:130],
                        pt2[:, 512 + q0 : 1024],
                        start=(k == 0),
                        stop=(k == nk - 1),
                    )
                    pt_tiles[k] = None

                for k in range(nk):
                    scores(k)
                    if k == 1 and pending is not None:
                        pending[0]()  # previous block's last two AVs
                    if k == 2 and pending is not None:
                        pending[1]()  # previous block's stage-out (must
                        # precede this block's av(0): same PSUM banks)
                    if feed:
                        feed.pop(0)()
                    if k >= 2:
                        av(k - 2)
                for t in feed:
                    t()

                outv = out.rearrange("h (nl q) -> h nl q", nl=HPC)

                def tail():
                    av(nk - 2)
                    av(nk - 1)

                def epilogue():
                    stage = pc.tile([65, 2, 512], F32, tag="stage", name="stage")
                    nc.vector.tensor_copy(stage[:, 0, :], oa[:])
                    nc.vector.tensor_copy(stage[:, 1, :], ob[:])
                    nc.sync.dma_start(
                        out=outv[:, 2 * p : 2 * p + 2, qb * 512 : (qb + 1) * 512],
                        in_=stage[:],
                    )

                if not last:
                    return (tail, epilogue)
                # last block: columns 0:384 are final after av(nk-2), so their
                # stage+DMA overlap av(nk-1) and only a 128-col sliver remains
                # after the very last matmul.  (A deeper 3-way split loses:
                # Tile's bank tracker serializes the early DVE reads against
                # the still-accumulating AV writes in the same PSUM bank.)
                av(nk - 2)
                stage = pc.tile([65, 2, 512], F32, tag="stage", name="stage")
                nc.vector.tensor_copy(stage[:, 0, 0:384], oa[:, 0:384])
                nc.vector.tensor_copy(stage[:, 1, 0:384], ob[:, 0:384])
                nc.sync.dma_start(
                    out=outv[:, 2 * p : 2 * p + 2, qb * 512 : qb * 512 + 384],
                    in_=stage[:, :, 0:384],
                )
                av(nk - 1)
                nc.vector.tensor_copy(stage[:, 0, 384:512], oa[:, 384:512])
                nc.vector.tensor_copy(stage[:, 1, 384:512], ob[:, 384:512])
                nc.scalar.dma_start(
                    out=outv[:, 2 * p : 2 * p + 2, qb * 512 + 384 : (qb + 1) * 512],
                    in_=stage[:, :, 384:512],
                )
                return None

            # prologue: just enough projection for the first attention block,
            # so the ScalarE exp stream (the span floor) starts ~7us in.
            qk_half(0, 0, 0)()  # K pair0 cols 0:512
            qk_half(0, 1, 0)()  # Q pair0 cols 0:512
            # static schedule: pair-0's four q-blocks run first so pair-1's
            # projections spread over the LONG late blocks (B02/B03 have
            # 12/16 k-steps) instead of piling into the short early ones;
            # every thunk still lands >= a few k-steps before first use.
            ep = attention(0, 0, [qk_half(0, 1, 1), v_chunk(0), v_chunk(1),
                                  v_chunk(2)])
            ep = attention(0, 1, [v_chunk(3), qk_half(0, 0, 1), v_chunk(4),
                                  v_chunk(5), v_chunk(6), v_chunk(7),
                                  qk_half(0, 1, 2)], ep)
            ep = attention(0, 2, [qk_half(0, 0, 2), v_chunk(8), v_chunk(9),
                                  v_chunk(10), v_chunk(11), qk_half(0, 1, 3)], ep)
            ep = attention(0, 3, [qk_half(0, 0, 3), v_chunk(12), v_chunk(13),
                                  v_chunk(14), v_chunk(15), qk_half(1, 1, 0),
                                  qk_half(1, 0, 0)], ep)
            ep = attention(1, 0, [qk_half(1, 1, 1), qk_half(1, 0, 1)], ep)
            ep = attention(1, 1, [qk_half(1, 1, 2)], ep)
            ep = attention(1, 2, [qk_half(1, 0, 2), qk_half(1, 1, 3)], ep)
            ep = attention(1, 3, [qk_half(1, 0, 3)], ep, last=True)

    _split_excess_waits(nc)
    return nc


_NC_CACHE = None


def _get_nc():
    global _NC_CACHE
    if _NC_CACHE is None:
        _NC_CACHE = build_nc()
    return _NC_CACHE


def _host_inputs(x, W_qkv):
    """Per-core input maps (fp16, partition-major DMA-friendly layouts)."""
    # xt [128p, c4, dt, 512]: d = dt*128 + p, s = c4*512 + col
    xT = [
        np.ascontiguousarray(
            x[b].T.reshape(DT, 128, NC4, 512).transpose(1, 2, 0, 3)
        ).astype(NPD)
        for b in range(B)
    ]
    ki = np.arange(128)[:, None]
    qj = np.arange(512)[None, :]
    m1 = np.zeros((4, 128, 512), dtype=np.float32)
    for r in range(4):
        m1[r] = (ki <= qj - 128 * r).astype(np.float32)
    mask = np.concatenate([m1, m1], axis=2)  # [4, 128, 1024]
    mask_h = np.ascontiguousarray(mask.transpose(1, 0, 2)).astype(NPD)

    # W_qkv[n, d, c]: q cols 0:64, k 64:128, v 128:192
    Wr = W_qkv.reshape(NH, DT, 128, 3 * HD)  # [n, dt, p, c]
    in_maps = []
    for c in range(NCORES):
        b = c // 4
        h0 = 4 * (c % 4)
        wqk = np.empty((128, 4 * DT, 128), dtype=np.float32)
        for p in range(NPAIR):
            ha, hb = h0 + 2 * p, h0 + 2 * p + 1
            for t in range(2):  # 0 = K, 1 = Q
                csl = slice(64, 128) if t == 0 else slice(0, 64)
                idx = (p * 2 + t) * DT
                wqk[:, idx : idx + DT, 0:64] = Wr[ha, :, :, csl].transpose(1, 0, 2)
                wqk[:, idx : idx + DT, 64:128] = Wr[hb, :, :, csl].transpose(1, 0, 2)
        wv = np.empty((128, DT, HPC * HD), dtype=np.float32)
        for h in range(HPC):
            wv[:, :, h * HD : (h + 1) * HD] = Wr[h0 + h, :, :, 128:192].transpose(
                1, 0, 2
            )
        in_maps.append(
            {
                "xt": xT[b],
                "wqk": wqk.astype(NPD),
                "wv": wv.astype(NPD),
                "mask": mask_h,
            }
        )
    return in_maps


def _host_epilogue(results, W_out):
    W_sum = W_out.sum(axis=(0, 1)).astype(np.float32)  # [D]
    O = np.empty((B, NH, S, HD), dtype=np.float32)
    for c in range(NCORES):
        o = results[c]["out"]  # [65, 4*2048]
        b = c // 4
        h0 = 4 * (c % 4)
        body = o[0:64].reshape(64, HPC, S)  # [h, nl, s]
        den = o[64].reshape(HPC, S)  # [nl, s]
        O[b, h0 : h0 + HPC] = body.transpose(1, 2, 0) / den[:, :, None]
    out2 = O.reshape(B, D, S)  # raw row-major reshape, as in the reference
    return np.ascontiguousarray(
        out2.transpose(0, 2, 1) * W_sum[None, None, :]
    ).astype(np.float32)


def _run(x, W_qkv, W_out, trace=False):
    nc = _get_nc()
    in_maps = _host_inputs(x, W_qkv)
    res = run_bass_kernel_spmd(
        nc,
        in_maps,
        list(range(NCORES)),
        trace=trace,
        trace_cores=list(range(NCORES)) if trace else None,
    )
    return _host_epilogue(res.results, W_out), res


def kernel(x, W_qkv, W_out):
    x = np.asarray(x, dtype=np.float32)
    W_qkv = np.asarray(W_qkv, dtype=np.float32)
    W_out = np.asarray(W_out, dtype=np.float32)
    out, _ = _run(x, W_qkv, W_out, trace=False)
    return out


def kernel_traced(x, W_qkv, W_out):
    out, res = _run(
        np.asarray(x, np.float32),
        np.asarray(W_qkv, np.float32),
        np.asarray(W_out, np.float32),
        trace=True,
    )
    return out, res
